# revision 21
# baseline (speedup 1.0000x reference)
"""Trainium2 kernel for nn_ApplyPolicyMap (lc0 policy-map apply).

out = reshape(x, [B, 5120]) @ fc1, where fc1 is a fixed 0/1 selection
matrix: every one of the 1858 output columns selects exactly one of the
5120 input features.  So the whole op is a feature gather:
    out[b, m] = x_flat[b, src_idx[m]],   src_idx = argmax(fc1, axis=0)

Distribution: shard x along the FEATURE dim, with cut points chosen so
every core owns ~1858/8 = 232..233 of the selected features (balanced
gather work).  The host stages each core's shard TRANSPOSED and cast to
bf16 (layout/dtype-only transform): xT [W, 16384], W = max shard width.
With features as DRAM rows the op becomes a row gather of 32KB
contiguous rows — ideal for DMA; no compute engine runs at all.

On-device per core:
  idx load:  one tiny int32 tile [128, 2G] (indices interleaved so each
             partition's values are contiguous -> 128 small descriptors).
  gather:    gpsimd indirect_dma_start pulls only the needed rows from
             HBM into SBUF partitions; padded index slots are
             out-of-bounds and skipped (no HBM read).  Each 128-row
             group is split into two 8192-column pieces (xT viewed as
             [2W, 8192], indices scaled 2*loc+c) so writes of early
             pieces overlap later gathers and the shared ~433 GB/s
             SBUF-AXI pipe never drains.
  store:     plain HWDGE DMAs of [128, 8192] pieces to the DRAM output,
             already in move-major order.
All DMAs span exactly 128 partitions: partial-partition DMAs collapse
onto 4 of 16 SDMA engines (trace-measured 108 vs 433 GB/s).
Per-core HBM traffic: ~7.6MB read + 8.4MB write (23 padded garbage rows
keep the store partition-full; host ignores them).
Host reassembles [B, 1858] f32 by transposing each core's gathered rows
into their final move columns.  Total error = bf16 quantization of x.
"""

import os

import ml_dtypes
import numpy as np

import concourse.bass as bass
import concourse.tile as tile
from concourse import bacc, mybir
from concourse.bass_utils import run_bass_kernel_spmd

N_CORES = 8
B = 16384
PLANES = 80
FLAT = PLANES * 64            # 5120
N_MOVES = 1858
NP = 2                        # column pieces per 128-row group
CB = B // NP                  # 8192 columns per piece
OOB_IDX = 1 << 20             # padding index; > bounds_check -> skipped

F32 = mybir.dt.float32
BF16 = mybir.dt.bfloat16
I32 = mybir.dt.int32

# Set by test harness to capture a neuron profile.
TRACE = bool(int(os.environ.get("KERNEL_TRACE", "0")))
TRACE_DIR = os.environ.get("KERNEL_TRACE_DIR") or None
LAST_RESULTS = None  # BassKernelResults of the most recent run (for profiling)


def _build_bass(n_groups, w):
    nc = bacc.Bacc("TRN2", target_bir_lowering=False, debug=False)

    # xT [w, B] viewed as [NP*w, CB]: row r of the view = column piece
    # r%NP of feature r//NP.  Gather indices are pre-scaled on host.
    xT = nc.dram_tensor("xT", [NP * w, CB], BF16, kind="ExternalInput").ap()
    idx = nc.dram_tensor("idx", [128, n_groups * NP], I32, kind="ExternalInput").ap()
    out = nc.dram_tensor("out", [n_groups * 128, B], BF16, kind="ExternalOutput").ap()

    with tile.TileContext(nc) as tc:
        with (
            tc.tile_pool(name="const", bufs=1) as cpool,
            tc.tile_pool(name="gbuf", bufs=2 * n_groups * NP) as gpool,
        ):
            # idx load on gpsimd: keeps the idx -> descriptor-gen chain on
            # one engine (no cross-engine semaphore hop before the gathers)
            idx_t = cpool.tile([128, n_groups * NP], I32, name="idx", tag="idx")
            nc.gpsimd.dma_start(idx_t[:], idx[:])
            tiles = {}
            for g in range(n_groups):
                for c in range(NP):
                    gt = gpool.tile([128, CB], BF16, name=f"g{g}_{c}", tag="g")
                    tiles[g, c] = gt
                    j = g * NP + c
                    nc.gpsimd.indirect_dma_start(
                        out=gt[:],
                        out_offset=None,
                        in_=xT[:],
                        in_offset=bass.IndirectOffsetOnAxis(
                            ap=idx_t[:, j : j + 1], axis=0
                        ),
                        bounds_check=NP * w - 1,
                        oob_is_err=False,
                    )
            for g in range(n_groups):
                for c in range(NP):
                    nc.sync.dma_start(
                        out[g * 128 : (g + 1) * 128, c * CB : (c + 1) * CB],
                        tiles[g, c][:],
                    )

    nc.compile()
    return nc


_NC_CACHE = {}


def _get_nc(n_groups, w):
    key = (n_groups, w)
    if key not in _NC_CACHE:
        _NC_CACHE[key] = _build_bass(n_groups, w)
    return _NC_CACHE[key]


def _make_policy_map_idx():
    # Deterministic stand-in policy map from the reference (seed 0).
    rng = np.random.RandomState(0)
    return rng.permutation(FLAT)[:N_MOVES].astype(np.int64)


def kernel(x, fc1=None):
    global LAST_RESULTS
    x = np.asarray(x, dtype=np.float32)
    x_flat = x.reshape(B, FLAT)
    if fc1 is not None:
        src_idx = np.argmax(np.asarray(fc1), axis=0).astype(np.int64)
    else:
        src_idx = _make_policy_map_idx()

    # Balanced feature-shard cuts: each core owns ~N_MOVES/8 selected rows.
    n = len(src_idx)
    ssorted = np.sort(src_idx)
    base, extra = divmod(n, N_CORES)
    counts_t = [base + (1 if i < extra else 0) for i in range(N_CORES)]
    cuts = [0]
    pos = 0
    for i in range(N_CORES - 1):
        pos += counts_t[i]
        cuts.append(int(ssorted[pos - 1] + ssorted[pos]) // 2 + 1)
    cuts.append(FLAT)

    w = max(cuts[i + 1] - cuts[i] for i in range(N_CORES))
    cap = max(counts_t)
    n_groups = (cap + 127) // 128

    # bf16 cast once, then per-core transposed shards (layout-only).
    x_bf = x_flat.astype(ml_dtypes.bfloat16)

    in_maps = []
    placement = []  # final move columns per core, in gathered-row order
    for i in range(N_CORES):
        lo, hi = cuts[i], cuts[i + 1]
        moves = np.where((src_idx >= lo) & (src_idx < hi))[0]
        loc = (src_idx[moves] - lo).astype(np.int64)
        order = np.argsort(loc, kind="stable")  # sequential HBM reads
        loc = loc[order]
        placement.append(moves[order])
        # interleaved, pre-scaled gather indices: idx[p, g*NP+c] selects
        # view-row NP*loc[g*128+p] + c
        idx_i = np.full((128, n_groups * NP), OOB_IDX, dtype=np.int32)
        for g in range(n_groups):
            rows = loc[g * 128 : (g + 1) * 128]
            for c in range(NP):
                idx_i[: len(rows), g * NP + c] = NP * rows + c
        xT_i = np.empty((w, B), dtype=ml_dtypes.bfloat16)
        xT_i[: hi - lo] = x_bf[:, lo:hi].T
        in_maps.append({"xT": xT_i.reshape(NP * w, CB), "idx": idx_i})

    nc = _get_nc(n_groups, w)
    if TRACE and TRACE_DIR and os.path.isdir(TRACE_DIR):
        # Stale NTFF/json artifacts from a previous traced run break the
        # profile conversion (duplicate model_index -> same json path).
        for f in os.listdir(TRACE_DIR):
            if f.endswith((".ntff", ".json", ".ntrc", ".pftrace")):
                try:
                    os.remove(os.path.join(TRACE_DIR, f))
                except OSError:
                    pass
    res = None
    for attempt in range(3):
        try:
            res = run_bass_kernel_spmd(
                nc, in_maps, core_ids=list(range(N_CORES)), trace=TRACE, tmpdir=TRACE_DIR
            )
            break
        except Exception:
            # Rare transient NRT_EXEC_UNIT_UNRECOVERABLE on first exec of a
            # freshly compiled NEFF; retry.
            import traceback as _tb

            _tb.print_exc()
            if attempt == 2:
                raise
            import time as _time

            _time.sleep(2.0)
    LAST_RESULTS = res

    out_full = np.empty((B, N_MOVES), dtype=np.float32)
    for i in range(N_CORES):
        fcols = placement[i]
        out_full[:, fcols] = res.results[i]["out"][: len(fcols)].T.astype(np.float32)
    return out_full


# revision 22
# speedup vs baseline: 1.0155x; 1.0155x over previous
"""Trainium2 kernel for nn_ApplyPolicyMap (lc0 policy-map apply).

out = reshape(x, [B, 5120]) @ fc1, where fc1 is a fixed 0/1 selection
matrix: every one of the 1858 output columns selects exactly one of the
5120 input features.  So the whole op is a feature gather:
    out[b, m] = x_flat[b, src_idx[m]],   src_idx = argmax(fc1, axis=0)

Distribution: shard x along the FEATURE dim, with cut points chosen so
every core owns ~1858/8 = 232..233 of the selected features (balanced
gather work).  The host stages each core's shard TRANSPOSED and cast to
bf16 (layout/dtype-only transform): xT [W, 16384], W = max shard width.
With features as DRAM rows the op becomes a row gather of 32KB
contiguous rows — ideal for DMA; no compute engine runs at all.

On-device per core:
  idx load:  one tiny int32 tile [128, 2G] (indices interleaved so each
             partition's values are contiguous -> 128 small descriptors).
  gather:    gpsimd indirect_dma_start pulls only the needed rows from
             HBM into SBUF partitions; padded index slots are
             out-of-bounds and skipped (no HBM read).  Each 128-row
             group is split into two 8192-column pieces (xT viewed as
             [2W, 8192], indices scaled 2*loc+c) so writes of early
             pieces overlap later gathers and the shared ~433 GB/s
             SBUF-AXI pipe never drains.
  store:     plain HWDGE DMAs of [128, 8192] pieces to the DRAM output,
             already in move-major order.
All DMAs span exactly 128 partitions: partial-partition DMAs collapse
onto 4 of 16 SDMA engines (trace-measured 108 vs 433 GB/s).
Per-core HBM traffic: ~7.6MB read + 8.4MB write (23 padded garbage rows
keep the store partition-full; host ignores them).
Host reassembles [B, 1858] f32 by transposing each core's gathered rows
into their final move columns.  Total error = bf16 quantization of x.
"""

import os

import ml_dtypes
import numpy as np

import concourse.bass as bass
import concourse.tile as tile
from concourse import bacc, mybir
from concourse.bass_utils import run_bass_kernel_spmd

N_CORES = 8
B = 16384
PLANES = 80
FLAT = PLANES * 64            # 5120
N_MOVES = 1858
NP = 2                        # column pieces per 128-row group
CB = B // NP                  # 8192 columns per piece
OOB_IDX = 1 << 20             # padding index; > bounds_check -> skipped

F32 = mybir.dt.float32
BF16 = mybir.dt.bfloat16
I32 = mybir.dt.int32

# Set by test harness to capture a neuron profile.
TRACE = bool(int(os.environ.get("KERNEL_TRACE", "0")))
TRACE_DIR = os.environ.get("KERNEL_TRACE_DIR") or None
LAST_RESULTS = None  # BassKernelResults of the most recent run (for profiling)


def _build_bass(n_groups, w):
    nc = bacc.Bacc("TRN2", target_bir_lowering=False, debug=False)

    # xT [w, B] viewed as [NP*w, CB]: row r of the view = column piece
    # r%NP of feature r//NP.  Gather indices are pre-scaled on host.
    xT = nc.dram_tensor("xT", [NP * w, CB], BF16, kind="ExternalInput").ap()
    idx = nc.dram_tensor("idx", [128, n_groups * NP], I32, kind="ExternalInput").ap()
    out = nc.dram_tensor("out", [n_groups * 128, B], BF16, kind="ExternalOutput").ap()

    with tile.TileContext(nc) as tc:
        with (
            tc.tile_pool(name="const", bufs=1) as cpool,
            tc.tile_pool(name="gbuf", bufs=2 * n_groups * NP) as gpool,
        ):
            idx_t = cpool.tile([128, n_groups * NP], I32, name="idx", tag="idx")
            nc.sync.dma_start(idx_t[:], idx[:])
            tiles = {}
            for g in range(n_groups):
                for c in range(NP):
                    gt = gpool.tile([128, CB], BF16, name=f"g{g}_{c}", tag="g")
                    tiles[g, c] = gt
                    j = g * NP + c
                    nc.gpsimd.indirect_dma_start(
                        out=gt[:],
                        out_offset=None,
                        in_=xT[:],
                        in_offset=bass.IndirectOffsetOnAxis(
                            ap=idx_t[:, j : j + 1], axis=0
                        ),
                        bounds_check=NP * w - 1,
                        oob_is_err=False,
                    )
            for g in range(n_groups):
                for c in range(NP):
                    nc.sync.dma_start(
                        out[g * 128 : (g + 1) * 128, c * CB : (c + 1) * CB],
                        tiles[g, c][:],
                    )

    nc.compile()
    return nc


_NC_CACHE = {}


def _get_nc(n_groups, w):
    key = (n_groups, w)
    if key not in _NC_CACHE:
        _NC_CACHE[key] = _build_bass(n_groups, w)
    return _NC_CACHE[key]


def _make_policy_map_idx():
    # Deterministic stand-in policy map from the reference (seed 0).
    rng = np.random.RandomState(0)
    return rng.permutation(FLAT)[:N_MOVES].astype(np.int64)


def kernel(x, fc1=None):
    global LAST_RESULTS
    x = np.asarray(x, dtype=np.float32)
    x_flat = x.reshape(B, FLAT)
    if fc1 is not None:
        src_idx = np.argmax(np.asarray(fc1), axis=0).astype(np.int64)
    else:
        src_idx = _make_policy_map_idx()

    # Balanced feature-shard cuts: each core owns ~N_MOVES/8 selected rows.
    n = len(src_idx)
    ssorted = np.sort(src_idx)
    base, extra = divmod(n, N_CORES)
    counts_t = [base + (1 if i < extra else 0) for i in range(N_CORES)]
    cuts = [0]
    pos = 0
    for i in range(N_CORES - 1):
        pos += counts_t[i]
        cuts.append(int(ssorted[pos - 1] + ssorted[pos]) // 2 + 1)
    cuts.append(FLAT)

    w = max(cuts[i + 1] - cuts[i] for i in range(N_CORES))
    cap = max(counts_t)
    n_groups = (cap + 127) // 128

    # bf16 cast once, then per-core transposed shards (layout-only).
    x_bf = x_flat.astype(ml_dtypes.bfloat16)

    in_maps = []
    placement = []  # final move columns per core, in gathered-row order
    for i in range(N_CORES):
        lo, hi = cuts[i], cuts[i + 1]
        moves = np.where((src_idx >= lo) & (src_idx < hi))[0]
        loc = (src_idx[moves] - lo).astype(np.int64)
        order = np.argsort(loc, kind="stable")  # sequential HBM reads
        loc = loc[order]
        placement.append(moves[order])
        # interleaved, pre-scaled gather indices: idx[p, g*NP+c] selects
        # view-row NP*loc[g*128+p] + c
        idx_i = np.full((128, n_groups * NP), OOB_IDX, dtype=np.int32)
        for g in range(n_groups):
            rows = loc[g * 128 : (g + 1) * 128]
            for c in range(NP):
                idx_i[: len(rows), g * NP + c] = NP * rows + c
        xT_i = np.empty((w, B), dtype=ml_dtypes.bfloat16)
        xT_i[: hi - lo] = x_bf[:, lo:hi].T
        in_maps.append({"xT": xT_i.reshape(NP * w, CB), "idx": idx_i})

    nc = _get_nc(n_groups, w)
    if TRACE and TRACE_DIR and os.path.isdir(TRACE_DIR):
        # Stale NTFF/json artifacts from a previous traced run break the
        # profile conversion (duplicate model_index -> same json path).
        for f in os.listdir(TRACE_DIR):
            if f.endswith((".ntff", ".json", ".ntrc", ".pftrace")):
                try:
                    os.remove(os.path.join(TRACE_DIR, f))
                except OSError:
                    pass
    res = None
    for attempt in range(3):
        try:
            res = run_bass_kernel_spmd(
                nc, in_maps, core_ids=list(range(N_CORES)), trace=TRACE, tmpdir=TRACE_DIR
            )
            break
        except Exception:
            # Rare transient NRT_EXEC_UNIT_UNRECOVERABLE on first exec of a
            # freshly compiled NEFF; retry.
            import traceback as _tb

            _tb.print_exc()
            if attempt == 2:
                raise
            import time as _time

            _time.sleep(2.0)
    LAST_RESULTS = res

    out_full = np.empty((B, N_MOVES), dtype=np.float32)
    for i in range(N_CORES):
        fcols = placement[i]
        out_full[:, fcols] = res.results[i]["out"][: len(fcols)].T.astype(np.float32)
    return out_full


# revision 24
# speedup vs baseline: 1.0273x; 1.0116x over previous
"""Trainium2 kernel for nn_ApplyPolicyMap (lc0 policy-map apply).

out = reshape(x, [B, 5120]) @ fc1, where fc1 is a fixed 0/1 selection
matrix: every one of the 1858 output columns selects exactly one of the
5120 input features.  So the whole op is a feature gather:
    out[b, m] = x_flat[b, src_idx[m]],   src_idx = argmax(fc1, axis=0)

Distribution: shard x along the FEATURE dim, with cut points chosen so
every core owns ~1858/8 = 232..233 of the selected features (balanced
gather work).  The host stages each core's shard TRANSPOSED and cast to
bf16 (layout/dtype-only transform): xT [W, 16384], W = max shard width.
With features as DRAM rows the op becomes a row gather of 32KB
contiguous rows — ideal for DMA; no compute engine runs at all.

On-device per core:
  idx load:  one tiny int32 tile [128, 2G] (indices interleaved so each
             partition's values are contiguous -> 128 small descriptors).
  gather:    gpsimd indirect_dma_start pulls only the needed rows from
             HBM into SBUF partitions; padded index slots are
             out-of-bounds and skipped (no HBM read).  Each 128-row
             group is split into two 8192-column pieces (xT viewed as
             [2W, 8192], indices scaled 2*loc+c) so writes of early
             pieces overlap later gathers and the shared ~433 GB/s
             SBUF-AXI pipe never drains.
  store:     plain HWDGE DMAs of [128, 8192] pieces to the DRAM output,
             already in move-major order.
All DMAs span exactly 128 partitions: partial-partition DMAs collapse
onto 4 of 16 SDMA engines (trace-measured 108 vs 433 GB/s).
Per-core HBM traffic: ~7.6MB read + 8.4MB write (23 padded garbage rows
keep the store partition-full; host ignores them).
Host reassembles [B, 1858] f32 by transposing each core's gathered rows
into their final move columns.  Total error = bf16 quantization of x.
"""

import os

import ml_dtypes
import numpy as np

import concourse.bass as bass
import concourse.tile as tile
from concourse import bacc, mybir
from concourse.bass_utils import run_bass_kernel_spmd

N_CORES = 8
B = 16384
PLANES = 80
FLAT = PLANES * 64            # 5120
N_MOVES = 1858
NP = 2                        # column pieces per 128-row group
CB = B // NP                  # 8192 columns per piece
OOB_IDX = 1 << 20             # padding index; > bounds_check -> skipped

F32 = mybir.dt.float32
BF16 = mybir.dt.bfloat16
I32 = mybir.dt.int32

# Set by test harness to capture a neuron profile.
TRACE = bool(int(os.environ.get("KERNEL_TRACE", "0")))
TRACE_DIR = os.environ.get("KERNEL_TRACE_DIR") or None
LAST_RESULTS = None  # BassKernelResults of the most recent run (for profiling)


def _build_bass(n_groups, w):
    nc = bacc.Bacc("TRN2", target_bir_lowering=False, debug=False)

    # xT [w, B] viewed as [NP*w, CB]: row r of the view = column piece
    # r%NP of feature r//NP.  Gather indices are pre-scaled on host.
    xT = nc.dram_tensor("xT", [NP * w, CB], BF16, kind="ExternalInput").ap()
    idx = nc.dram_tensor("idx", [128, n_groups * NP], I32, kind="ExternalInput").ap()
    out = nc.dram_tensor("out", [n_groups * 128, B], BF16, kind="ExternalOutput").ap()

    with tile.TileContext(nc) as tc:
        with (
            tc.tile_pool(name="const", bufs=1) as cpool,
            tc.tile_pool(name="gbuf", bufs=2 * n_groups * NP) as gpool,
        ):
            idx_t = cpool.tile([128, n_groups * NP], I32, name="idx", tag="idx")
            nc.sync.dma_start(idx_t[:], idx[:])
            tiles = {}
            for g in range(n_groups):
                for c in range(NP):
                    gt = gpool.tile([128, CB], BF16, name=f"g{g}_{c}", tag="g")
                    tiles[g, c] = gt
                    j = g * NP + c
                    nc.gpsimd.indirect_dma_start(
                        out=gt[:],
                        out_offset=None,
                        in_=xT[:],
                        in_offset=bass.IndirectOffsetOnAxis(
                            ap=idx_t[:, j : j + 1], axis=0
                        ),
                        bounds_check=NP * w - 1,
                        oob_is_err=False,
                    )
            for g in range(n_groups):
                for c in range(NP):
                    nc.sync.dma_start(
                        out[g * 128 : (g + 1) * 128, c * CB : (c + 1) * CB],
                        tiles[g, c][:],
                    )

    nc.compile()
    return nc


_NC_CACHE = {}


def _get_nc(n_groups, w):
    key = (n_groups, w)
    if key not in _NC_CACHE:
        _NC_CACHE[key] = _build_bass(n_groups, w)
    return _NC_CACHE[key]


def _make_policy_map_idx():
    # Deterministic stand-in policy map from the reference (seed 0).
    rng = np.random.RandomState(0)
    return rng.permutation(FLAT)[:N_MOVES].astype(np.int64)


def kernel(x, fc1=None):
    global LAST_RESULTS
    x = np.asarray(x, dtype=np.float32)
    x_flat = x.reshape(B, FLAT)
    if fc1 is not None:
        src_idx = np.argmax(np.asarray(fc1), axis=0).astype(np.int64)
    else:
        src_idx = _make_policy_map_idx()

    # Balanced feature-shard cuts: each core owns ~N_MOVES/8 selected rows.
    n = len(src_idx)
    ssorted = np.sort(src_idx)
    base, extra = divmod(n, N_CORES)
    counts_t = [base + (1 if i < extra else 0) for i in range(N_CORES)]
    cuts = [0]
    pos = 0
    for i in range(N_CORES - 1):
        pos += counts_t[i]
        cuts.append(int(ssorted[pos - 1] + ssorted[pos]) // 2 + 1)
    cuts.append(FLAT)

    w = max(cuts[i + 1] - cuts[i] for i in range(N_CORES))
    cap = max(counts_t)
    n_groups = (cap + 127) // 128

    # bf16 cast once, then per-core transposed shards (layout-only).
    x_bf = x_flat.astype(ml_dtypes.bfloat16)

    in_maps = []
    placement = []  # final move columns per core, in gathered-row order
    for i in range(N_CORES):
        lo, hi = cuts[i], cuts[i + 1]
        moves = np.where((src_idx >= lo) & (src_idx < hi))[0]
        loc = (src_idx[moves] - lo).astype(np.int64)
        order = np.argsort(loc, kind="stable")  # sequential HBM reads
        loc = loc[order]
        moves = moves[order]
        # interleaved, pre-scaled gather indices: idx[p, g*NP+c] selects
        # view-row NP*loc[g*128+p] + c.  In a partial group, spread the
        # OOB pad slots EVENLY across partitions (not clustered at the
        # end): each SDMA engine serves a fixed partition set, so
        # clustered pads make a few engines run short on read work and
        # idle during the tail while the rest still drain.
        idx_i = np.full((128, n_groups * NP), OOB_IDX, dtype=np.int32)
        out_rows, out_moves = [], []
        for g in range(n_groups):
            rows = loc[g * 128 : (g + 1) * 128]
            n_seg = len(rows)
            if n_seg == 128:
                parts = np.arange(128)
            else:
                n_pad = 128 - n_seg
                pad = set((np.arange(n_pad) * (128.0 / n_pad)).astype(int))
                parts = np.array([p for p in range(128) if p not in pad])
                parts = parts[:n_seg]
            for k, p in enumerate(parts):
                for c in range(NP):
                    idx_i[p, g * NP + c] = NP * rows[k] + c
                out_rows.append(g * 128 + int(p))
                out_moves.append(moves[g * 128 + k])
        placement.append((np.array(out_rows), np.array(out_moves)))
        xT_i = np.empty((w, B), dtype=ml_dtypes.bfloat16)
        xT_i[: hi - lo] = x_bf[:, lo:hi].T
        in_maps.append({"xT": xT_i.reshape(NP * w, CB), "idx": idx_i})

    nc = _get_nc(n_groups, w)
    if TRACE and TRACE_DIR and os.path.isdir(TRACE_DIR):
        # Stale NTFF/json artifacts from a previous traced run break the
        # profile conversion (duplicate model_index -> same json path).
        for f in os.listdir(TRACE_DIR):
            if f.endswith((".ntff", ".json", ".ntrc", ".pftrace")):
                try:
                    os.remove(os.path.join(TRACE_DIR, f))
                except OSError:
                    pass
    res = None
    for attempt in range(3):
        try:
            res = run_bass_kernel_spmd(
                nc, in_maps, core_ids=list(range(N_CORES)), trace=TRACE, tmpdir=TRACE_DIR
            )
            break
        except Exception:
            # Rare transient NRT_EXEC_UNIT_UNRECOVERABLE on first exec of a
            # freshly compiled NEFF; retry.
            import traceback as _tb

            _tb.print_exc()
            if attempt == 2:
                raise
            import time as _time

            _time.sleep(2.0)
    LAST_RESULTS = res

    out_full = np.empty((B, N_MOVES), dtype=np.float32)
    for i in range(N_CORES):
        out_rows, out_moves = placement[i]
        out_full[:, out_moves] = (
            res.results[i]["out"][out_rows].T.astype(np.float32)
        )
    return out_full
